# revision 1
# baseline (speedup 1.0000x reference)
"""AgentAttention Trainium2 kernel.

Data-parallel over batch: 32 samples -> 8 cores x 4 samples.
Device layout is channels-major ("transposed"): activations live as (c, t).

Per-sample pipeline (all on device):
  qk^T  = Wqk^T.T @ xs^T                  (fp32r matmuls)
  v_t   = xs^T.T @ Wv^T   (tokens-major, bf16 for A1V)
  v^T   = Wv^T.T @ xs^T   (channels-major, zero-padded 34x34 image for dwc)
  agents^T: strided-window sums of q^T on DVE (adaptive avg pool), scaled
  S1^T[t,(h,a)] = k^T.T @ blockdiag(agents)    -> exp on ACT -> *expB1 (gpsimd)
  A1V: agent_v[(a),(d)] + denominators via ones column; normalize
  S2[(h,a),t]  = blockdiag(agents).T @ q^T     -> exp on ACT -> *expB2 (DVE)
  A2V: out2 + denominators via ones columns; normalize with partition_broadcast
  dwc: 9 diagonal matmuls over shifted views of padded v^T
  proj: Wp^T.T @ pre_proj
Host adds proj/dwc biases and restores (b, n+1, c) order.
"""

import numpy as np
import ml_dtypes

DEBUG = False
STAGE = 99  # truncate pipeline for perf bisect
REPEAT = 0  # >0: wrap sample loop in a hardware For_i for timing
NOBCAST = False

import concourse.bacc as bacc
import concourse.tile as tile
import concourse.mybir as mybir
from concourse import bass_utils

N_CORES = 8
B = 32
SPB = B // N_CORES  # samples per core
C = 256
NT = 1024  # spatial tokens
WIN = 32
HEADS = 8
HD = 32
AGENT = 49
POOL = 7
SCALE = HD ** -0.5

F32 = mybir.dt.float32
F32R = mybir.dt.float32r
BF16 = mybir.dt.bfloat16
AF = mybir.ActivationFunctionType
ALU = mybir.AluOpType
AX = mybir.AxisListType

BINS_START = [(i * WIN) // POOL for i in range(POOL)]
BINS_END = [-((-(i + 1) * WIN) // POOL) for i in range(POOL)]


# ----------------------------------------------------------------- host prep
def _resize_bilinear_7_to_32(b):
    """jax.image.resize 'bilinear' (half-pixel) for trailing (7,7)->(32,32)."""
    src, dst = 7, 32
    coords = (np.arange(dst) + 0.5) * (src / dst) - 0.5
    i0 = np.floor(coords).astype(np.int64)
    frac = coords - i0
    i0c = np.clip(i0, 0, src - 1)
    i1c = np.clip(i0 + 1, 0, src - 1)

    def along(x, axis):
        a0 = np.take(x, i0c, axis=axis)
        a1 = np.take(x, i1c, axis=axis)
        sh = [1] * x.ndim
        sh[axis] = dst
        f = frac.reshape(sh)
        return a0 * (1.0 - f) + a1 * f

    return along(along(b, -2), -1)


def _host_consts(qkv_w, proj_w, proj_b, dwc_w, dwc_b,
                 an_bias, ah_bias, aw_bias, na_bias, ha_bias, wa_bias):
    c = {}
    c["wqk"] = np.ascontiguousarray(
        qkv_w[:2 * C].T.reshape(2, 128, 2 * C)).astype(np.float32)
    c["wv"] = np.ascontiguousarray(
        qkv_w[2 * C:].T.reshape(2, 128, C)).astype(np.float32)
    c["wp"] = np.ascontiguousarray(proj_w.T.reshape(2, 128, C)).astype(np.float32)

    # stage-1 bias, exp'ed, layout (t, 256*g + 49*h'' + a), pads -> exp(0)=1
    pb1 = _resize_bilinear_7_to_32(an_bias).reshape(HEADS, AGENT, NT)
    pb2 = (ah_bias + aw_bias).reshape(HEADS, AGENT, NT)
    b1 = pb1 + pb2  # (H, A, t)
    # head blocks 64-aligned: col = 256*g + 64*h'' + a  (a < 49, rest pad)
    eb1 = np.zeros((NT, 512), np.float32)
    for g in range(2):
        for hh in range(4):
            eb1[:, 256 * g + 64 * hh:256 * g + 64 * hh + AGENT] = \
                b1[4 * g + hh].T
    c["expB1"] = np.exp(eb1).reshape(NT // 128, 128, 512).astype(ml_dtypes.bfloat16)

    # stage-2 bias, exp'ed, layout [pair][49*e + a, t]
    ab1 = _resize_bilinear_7_to_32(na_bias).reshape(HEADS, AGENT, NT)  # [h,a,t]
    ha = ha_bias.reshape(HEADS, AGENT, WIN)      # [h, a, r]  (from (1,H,A,WIN,1))
    wa = wa_bias.reshape(HEADS, AGENT, WIN)      # [h, a, w]  (from (1,H,A,1,WIN))
    # bias2[h, t=(r,w), a] = ab1[h,a,t] + ha_bias[h,r,a] + wa_bias[h,w,a]
    # NOTE ha_bias is (1,H,A,WIN,1): reshape -> [h, a, r]; wa -> [h, a, w]
    b2 = (ab1.reshape(HEADS, AGENT, WIN, WIN)
          + ha[:, :, :, None] + wa[:, :, None, :]).reshape(HEADS, AGENT, NT)
    # rows 64-aligned: row = 64*e + a; dead rows (49:64) stay 0 so they
    # contribute nothing through the zero columns of BDagv.
    eb2 = np.zeros((4, 113, NT), np.float32)
    for p in range(4):
        for e in range(2):
            eb2[p, 64 * e:64 * e + AGENT] = np.exp(b2[2 * p + e])
    c["expB2"] = eb2.astype(ml_dtypes.bfloat16)

    # pooled-agent scale (fold pool mean + attention scale), replicated rows
    sz = np.array([BINS_END[i] - BINS_START[i] for i in range(POOL)], np.float32)
    sa = SCALE / (sz[:, None] * sz[None, :])  # [i, j]
    c["sa"] = np.broadcast_to(sa.reshape(1, AGENT), (128, AGENT)).astype(np.float32).copy()

    # dwc diagonal blocks: slot tap*2+chunk, tap = 3*dr+dc
    w3 = dwc_w.reshape(C, 3, 3).astype(np.float32)
    w3d = np.zeros((18, 128, 128), np.float32)
    for tap in range(9):
        dr, dc = tap // 3, tap % 3
        for ci in range(2):
            np.fill_diagonal(w3d[tap * 2 + ci], w3[128 * ci:128 * ci + 128, dr, dc])
    c["w3d"] = w3d

    # device zero-fill sources and the ones pattern for odd-pair denominators
    c["zz"] = np.zeros((128, 2312), np.float32)
    ob = np.zeros((128, 33), np.float32)
    ob[0:49, 0] = 1.0
    ob[64:113, 32] = 1.0
    ob[:, 1] = 1.0  # col 1: all-ones (for v_t ones column)
    c["onesBD"] = ob.astype(ml_dtypes.bfloat16)
    zb = np.zeros((128, 512), np.float32)
    c["zzb"] = zb.astype(ml_dtypes.bfloat16)

    # host-side output biases
    c["bias_cls"] = proj_b.astype(np.float32)
    c["bias_sp"] = (proj_b + proj_w @ dwc_b).astype(np.float32)
    return c




def _mm512(nc, out, lhsT, rhs, start, stop, n):
    """f32r matmul with the moving free dim split at 512 elements."""
    for n0 in range(0, n, 512):
        n1 = min(n0 + 512, n)
        nc.tensor.matmul(out[:, n0:n1], lhsT, rhs[:, n0:n1],
                         start=start, stop=stop)


# ------------------------------------------------------------- device build
def build_nc():
    nc = bacc.Bacc("TRN2", target_bir_lowering=False, debug=False,
                   num_devices=N_CORES)
    dr = {}
    dr["xT"] = nc.dram_tensor("xT", (SPB, 2, 128, NT + 1), F32R,
                              kind="ExternalInput").ap()
    dr["wqk"] = nc.dram_tensor("wqk", (2, 128, 512), F32R, kind="ExternalInput").ap()
    dr["wv"] = nc.dram_tensor("wv", (2, 128, 256), F32R, kind="ExternalInput").ap()
    dr["wp"] = nc.dram_tensor("wp", (2, 128, 256), F32R, kind="ExternalInput").ap()
    dr["expB1"] = nc.dram_tensor("expB1", (8, 128, 512), BF16, kind="ExternalInput").ap()
    dr["expB2"] = nc.dram_tensor("expB2", (4, 113, NT), BF16, kind="ExternalInput").ap()
    dr["sa"] = nc.dram_tensor("sa", (128, AGENT), F32, kind="ExternalInput").ap()
    dr["w3d"] = nc.dram_tensor("w3d", (18, 128, 128), F32R, kind="ExternalInput").ap()
    dr["zz"] = nc.dram_tensor("zz", (128, 2312), F32R, kind="ExternalInput").ap()
    dr["onesBD"] = nc.dram_tensor("onesBD", (128, 33), BF16, kind="ExternalInput").ap()
    dr["zzb"] = nc.dram_tensor("zzb", (128, 512), BF16, kind="ExternalInput").ap()
    dr["scr"] = nc.dram_tensor("scr", (SPB, 4, 2, NT), F32, kind="Internal").ap()
    dr["y"] = nc.dram_tensor("y", (SPB, 2, 128, NT + 1), F32,
                             kind="ExternalOutput").ap()
    if DEBUG:
        dr["d_qkT"] = nc.dram_tensor("d_qkT", (128, 4, NT), F32R, kind="ExternalOutput").ap()
        dr["d_vt"] = nc.dram_tensor("d_vt", (128, 8, 4, 65), BF16, kind="ExternalOutput").ap()
        dr["d_AG"] = nc.dram_tensor("d_AG", (128, 2, AGENT), F32R, kind="ExternalOutput").ap()
        dr["d_eS1"] = nc.dram_tensor("d_eS1", (128, 8, 512), BF16, kind="ExternalOutput").ap()
        dr["d_agv"] = nc.dram_tensor("d_agv", (128, 4, 97), BF16, kind="ExternalOutput").ap()
        dr["d_eS2"] = nc.dram_tensor("d_eS2", (128, 4, NT), BF16, kind="ExternalOutput").ap()
        dr["d_pre"] = nc.dram_tensor("d_pre", (128, 2, NT + 1), F32R, kind="ExternalOutput").ap()
        dr["d_vTp"] = nc.dram_tensor("d_vTp", (128, 2, 34, 34), F32R, kind="ExternalOutput").ap()
        dr["d_rb"] = nc.dram_tensor("d_rb", (4, 128, NT), F32, kind="ExternalOutput").ap()
        dr["d_r2"] = nc.dram_tensor("d_r2", (4, 128, NT), F32, kind="ExternalOutput").ap()
        dr["d_pre2"] = nc.dram_tensor("d_pre2", (128, 2, NT + 1), F32R, kind="ExternalOutput").ap()

    with tile.TileContext(nc) as tc:
        _emit(tc, dr)
    nc.compile()
    return nc


def _emit(tc, dr):
    nc = tc.nc
    from contextlib import ExitStack
    with ExitStack() as ctx:
        cpool = ctx.enter_context(tc.tile_pool(name="consts", bufs=1))
        sp1 = ctx.enter_context(tc.tile_pool(name="sp1", bufs=1))
        sp2 = ctx.enter_context(tc.tile_pool(name="sp2", bufs=2))
        sps = ctx.enter_context(tc.tile_pool(name="sps", bufs=2))
        spq = ctx.enter_context(tc.tile_pool(name="spq", bufs=3))
        ps_big = ctx.enter_context(
            tc.tile_pool(name="ps_big", bufs=2, space="PSUM"))
        ps_a = ctx.enter_context(tc.tile_pool(name="ps_a", bufs=3, space="PSUM"))
        ps_dwc = ctx.enter_context(tc.tile_pool(name="ps_dwc", bufs=1, space="PSUM"))

        # ---- constants to SBUF
        wqk = cpool.tile([128, 2, 512], F32R)
        wv = cpool.tile([128, 2, 256], F32R)
        wp = cpool.tile([128, 2, 256], F32R)
        eB1 = cpool.tile([128, 8, 512], BF16)
        eB2 = cpool.tile([128, 4, NT], BF16)
        sa = cpool.tile([128, AGENT], F32)
        w3d = cpool.tile([128, 18, 128], F32R)
        for ki in range(2):
            nc.sync.dma_start(wqk[:, ki, :], dr["wqk"][ki])
            nc.sync.dma_start(wv[:, ki, :], dr["wv"][ki])
            nc.sync.dma_start(wp[:, ki, :], dr["wp"][ki])
        for ti in range(8):
            nc.sync.dma_start(eB1[:, ti, :], dr["expB1"][ti])
        for p in range(4):
            nc.sync.dma_start(eB2[0:113, p, :], dr["expB2"][p])
        nc.sync.dma_start(sa[:], dr["sa"][:])
        for s18 in range(18):
            nc.sync.dma_start(w3d[:, s18, :], dr["w3d"][s18])

        # persistent tiles whose zero regions are written exactly once
        vTp = cpool.tile([128, 2, 34, 34], F32R)
        BD1 = cpool.tile([128, 2, 256], F32R)
        BD2 = cpool.tile([128, 4, 113], F32R)
        BDagv = cpool.tile([128, 4, 97], BF16)
        onesBD = cpool.tile([128, 33], BF16)
        v_t = cpool.tile([128, 8, 4, 65], BF16)
        nc.sync.dma_start(vTp[:].rearrange("p a b c -> p (a b c)"), dr["zz"][:, 0:2312])
        nc.sync.dma_start(BD1[:].rearrange("p a b -> p (a b)"), dr["zz"][:, 0:512])
        nc.sync.dma_start(BD2[:].rearrange("p a b -> p (a b)"), dr["zz"][:, 0:452])
        nc.sync.dma_start(BDagv[:].rearrange("p a b -> p (a b)"), dr["zzb"][:, 0:388])
        nc.sync.dma_start(onesBD[:], dr["onesBD"][:])
        for ti in range(8):
            for p4 in range(4):
                nc.gpsimd.tensor_copy(v_t[:, ti, p4, 64:65], onesBD[:, 1:2])

        def body():
            for s in range(SPB):
                _sample(tc, dr, s, wqk, wv, wp, eB1, eB2, sa, w3d,
                        vTp, BD1, BD2, BDagv, onesBD, v_t,
                        sp1, sp2, sps, spq, ps_big, ps_a, ps_dwc)

        if REPEAT > 0:
            with tc.For_i(0, REPEAT, 1):
                body()
        else:
            body()


def _sample(tc, dr, s, wqk, wv, wp, eB1, eB2, sa, w3d,
            vTp, BD1, BD2, BDagv, onesBD, v_t,
            sp1, sp2, sps, spq, ps_big, ps_a, ps_dwc):
    nc = tc.nc

    # ---- load x^T (2 chunks of (128, 1025)); col 0 = cls token
    xT = sp2.tile([128, 2, NT + 1], F32R, tag="xT")
    for ci in range(2):
        nc.sync.dma_start(xT[:, ci, :], dr["xT"][s, ci])

    # ---- qk^T: 4 m-chunks (q: 0,1 / k: 2,3), accumulate over 2 k-chunks
    qkT = spq.tile([128, 4, NT], F32R, tag="qkT")
    for mi in range(4):
        acc = ps_big.tile([128, NT], F32, tag="big")
        for ki in range(2):
            _mm512(nc, acc, wqk[:, ki, 128 * mi:128 * mi + 128],
                   xT[:, ki, 1:NT + 1], ki == 0, ki == 1, NT)
        nc.scalar.activation(qkT[:, mi, :], acc[:], AF.Copy)

    # ---- v tokens-major (128t x 256c per chunk) -> bf16 (..., 4, 65) with ones col
    for ti in range(8):
        acc = ps_a.tile([128, 256], F32, tag="a")
        for ki in range(2):
            nc.tensor.matmul(acc[:], xT[:, ki, 1 + 128 * ti:1 + 128 * ti + 128],
                             wv[:, ki, :], start=(ki == 0), stop=(ki == 1))
        nc.vector.tensor_copy(
            v_t[:, ti, :, 0:64], acc[:].rearrange("p (a b) -> p a b", a=4))

    # ---- v^T into zero-padded (34,34) image per chunk
    for ci in range(2):
        acc = ps_big.tile([128, NT], F32, tag="big")
        for ki in range(2):
            _mm512(nc, acc, wv[:, ki, 128 * ci:128 * ci + 128],
                   xT[:, ki, 1:NT + 1], ki == 0, ki == 1, NT)
        nc.vector.tensor_copy(
            vTp[:, ci, 1:33, 1:33], acc[:].rearrange("p (h w) -> p h w", h=32))

    if DEBUG and s == 0:
        nc.sync.dma_start(dr["d_qkT"][:], qkT[:])
        nc.sync.dma_start(dr["d_vt"][:], v_t[:])
        nc.sync.dma_start(dr["d_vTp"][:], vTp[:])

    if STAGE < 2:
        return
    # ---- adaptive pool of q^T -> agents^T (AG), scaled
    RP = sps.tile([128, 2, POOL, WIN], F32, tag="RP")
    AGf = sps.tile([128, 2, AGENT], F32, tag="AGf")
    AG = sps.tile([128, 2, AGENT], F32R, tag="AG")
    for ci in range(2):
        qv = qkT[:, ci, :].rearrange("p (h w) -> p w h", h=WIN)  # (128, w, h)
        for i in range(POOL):
            nc.vector.reduce_sum(RP[:, ci, i, :],
                                 qv[:, :, BINS_START[i]:BINS_END[i]], axis=AX.X)
        agv = AGf[:, ci, :].rearrange("p (i j) -> p j i", j=POOL)  # (128, j, i)
        for j in range(POOL):
            nc.vector.reduce_sum(agv[:, j, :],
                                 RP[:, ci, :, BINS_START[j]:BINS_END[j]], axis=AX.X)
        nc.vector.tensor_tensor(AG[:, ci, :], AGf[:, ci, :], sa[:], op=ALU.mult)

    # ---- block-diagonal agent tiles
    for g in range(2):
        for hh in range(4):
            nc.gpsimd.tensor_copy(
                BD1[32 * hh:32 * hh + 32, g, 64 * hh:64 * hh + AGENT],
                AG[32 * hh:32 * hh + 32, g, :])
    for p in range(4):
        b = 64 * (p % 2)  # partition base of this pair's q^T rows
        for e in range(2):
            nc.gpsimd.tensor_copy(
                BD2[b + 32 * e:b + 32 * e + 32, p, 64 * e:64 * e + AGENT],
                AG[b + 32 * e:b + 32 * e + 32, p // 2, :])

    if DEBUG and s == 0:
        nc.sync.dma_start(dr["d_AG"][:], AG[:])

    # ---- stage 1 scores^T (t, (h,a)) + exp + bias factor
    expS1 = sp2.tile([128, 8, 512], BF16, tag="expS1")
    for ti in range(8):
        acc = ps_a.tile([128, 512], F32, tag="a")
        for g in range(2):
            nc.tensor.matmul(acc[:, 256 * g:256 * g + 256],
                             qkT[:, 2 + g, 128 * ti:128 * ti + 128],
                             BD1[:, g, :], start=True, stop=True)
        ev = expS1[:, ti, :].rearrange("p (h c) -> p h c", c=64)[:, :, 0:49]
        av = acc[:].rearrange("p (h c) -> p h c", c=64)[:, :, 0:49]
        bv = eB1[:, ti, :].rearrange("p (h c) -> p h c", c=64)[:, :, 0:49]
        nc.scalar.activation(ev, av, AF.Exp)
        nc.gpsimd.tensor_tensor(ev, ev, bv, op=ALU.mult)

    if STAGE < 3:
        return
    # ---- A1V: agent_v (pair-local rows 64e+a) + denominators
    # BDagv row layout: e0 at 0:49, e1 at 64:113 (dead zone keeps bases aligned)
    # BDagv cols: data at 0:64 for all pairs; even pairs also carry ones at
    # col 64 (rows 0:49) and col 96 (rows 64:113) so denominators land at
    # A2V out rows 64/96. Odd pairs get denominators from a second small
    # matmul against onesBD (ones at cols 0/32 -> out rows 0/32).
    rec = sps.tile([128, 4, 1], F32, tag="rec")
    for p in range(0, 4, 2):
        nc.gpsimd.tensor_copy(BDagv[0:49, p, 64:65], onesBD[0:49, 0:1])
        nc.gpsimd.tensor_copy(BDagv[64:113, p, 96:97], onesBD[64:113, 32:33])
    for p in range(4):
        acc = ps_a.tile([113, 65], F32, tag="a")
        c0 = 256 * (p // 2) + 128 * (p % 2)
        for ti in range(8):
            nc.tensor.matmul(acc[:], expS1[:, ti, c0:c0 + 113],
                             v_t[:, ti, p, :], start=(ti == 0), stop=(ti == 7))
        nc.vector.reciprocal(rec[0:113, p, :], acc[:, 64:65])
        for e in range(2):
            nc.vector.tensor_scalar(
                out=BDagv[64 * e:64 * e + 49, p, 32 * e:32 * e + 32],
                in0=acc[64 * e:64 * e + 49, 32 * e:32 * e + 32],
                scalar1=rec[64 * e:64 * e + 49, p, :],
                scalar2=None, op0=ALU.mult)

    if DEBUG and s == 0:
        nc.sync.dma_start(dr["d_eS1"][:], expS1[:])
        nc.sync.dma_start(dr["d_agv"][:], BDagv[:])

    if STAGE < 4:
        return
    # ---- stage 2 scores ((h,a), t) + exp + bias factor
    expS2 = sp2.tile([128, 4, NT], BF16, tag="expS2")
    for p in range(4):
        b = 64 * (p % 2)
        acc = ps_big.tile([113, NT], F32, tag="big")
        _mm512(nc, acc, BD2[b:b + 64, p, :],
               qkT[b:b + 64, p // 2, :], True, True, NT)
        nc.scalar.activation(expS2[0:113, p, :], acc[:], AF.Exp)
        nc.gpsimd.tensor_tensor(expS2[0:113, p, :], expS2[0:113, p, :],
                                 eB2[0:113, p, :], op=ALU.mult)

    # ---- A2V + normalization -> pre_proj cols 0:1024 (spatial), col 1024 = cls
    pre = sp2.tile([128, 2, NT + 1], F32R, tag="pre")
    for ci in range(2):
        nc.gpsimd.tensor_copy(pre[:, ci, NT:NT + 1], xT[:, ci, 0:1])
    for p in range(4):
        b = 64 * (p % 2)
        acc = ps_big.tile([128, NT], F32, tag="big")
        if p % 2 == 0:
            # one matmul: data rows 0:64, denominators at rows 64 / 96
            _mm512(nc, acc[0:97, :], BDagv[0:113, p, :],
                   expS2[0:113, p, :], True, True, NT)
            dn0 = 64
        else:
            # data -> rows 64:128; denominators -> rows 0 / 32
            _mm512(nc, acc[64:128, :], BDagv[0:113, p, 0:64],
                   expS2[0:113, p, :], True, True, NT)
            _mm512(nc, acc[0:33, :], onesBD[0:113, :],
                   expS2[0:113, p, :], True, True, NT)
            dn0 = 0
        r2 = sps.tile([128, NT], F32, tag="r2")
        rb = sps.tile([128, NT], F32, tag="rb")
        nc.vector.reciprocal(r2[dn0:dn0 + 33, :], acc[dn0:dn0 + 33, :])
        if not NOBCAST:
            for e in range(2):
                # HW partition_broadcast ignores the input partition offset, so
                # bounce the row off DRAM and re-load with 0-stride partition AP.
                nc.sync.dma_start(dr["scr"][s, p, e],
                                  r2[dn0 + 32 * e:dn0 + 32 * e + 1, :])
                nc.gpsimd.dma_start(rb[b + 32 * e:b + 32 * e + 32, :],
                                    dr["scr"][s, p, e].partition_broadcast(32))
        if DEBUG and s == 0:
            nc.sync.dma_start(dr["d_rb"][p], rb[:])
            nc.sync.dma_start(dr["d_r2"][p], r2[:])
        nc.vector.tensor_tensor(
            pre[b:b + 64, p // 2, 0:NT],
            acc[b:b + 64, :], rb[b:b + 64, :], op=ALU.mult)

    if DEBUG and s == 0:
        nc.sync.dma_start(dr["d_eS2"][:], expS2[:])
        nc.sync.dma_start(dr["d_pre2"][:], pre[:])

    if STAGE < 5:
        return
    # ---- dwc accumulated on top of pre_proj spatial cols
    for ci in range(2):
        for hf in range(2):
            acc = ps_dwc.tile([128, 512], F32, tag="dwc")
            accv = acc[:].rearrange("p (h w) -> p h w", h=16)
            for tap in range(9):
                dr_, dc_ = tap // 3, tap % 3
                nc.tensor.matmul(accv[:],
                                 w3d[:, tap * 2 + ci, :],
                                 vTp[:, ci, dr_ + 16 * hf:dr_ + 16 * hf + 16,
                                     dc_:dc_ + 32],
                                 start=(tap == 0), stop=(tap == 8))
            nc.vector.tensor_tensor(pre[:, ci, 512 * hf:512 * hf + 512],
                                    acc[:], pre[:, ci, 512 * hf:512 * hf + 512],
                                    op=ALU.add)

    if DEBUG and s == 0:
        nc.sync.dma_start(dr["d_pre"][:], pre[:])

    if STAGE < 6:
        return
    # ---- proj
    outT = sp2.tile([128, 2, NT + 1], F32, tag="outT")
    for mi in range(2):
        for n0, n1 in ((0, 512), (512, 1024), (1024, 1025)):
            acc = ps_a.tile([128, 512], F32, tag="a")
            for ki in range(2):
                if n1 - n0 == 1:
                    nc.tensor.matmul(acc[:, 0:1],
                                     wp[:, ki, 128 * mi:128 * mi + 128].bitcast(F32),
                                     pre[:, ki, n0:n1].bitcast(F32),
                                     start=(ki == 0), stop=(ki == 1))
                else:
                    nc.tensor.matmul(acc[:, 0:n1 - n0],
                                     wp[:, ki, 128 * mi:128 * mi + 128],
                                     pre[:, ki, n0:n1], start=(ki == 0), stop=(ki == 1))
            nc.scalar.activation(outT[:, mi, n0:n1], acc[:, 0:n1 - n0], AF.Copy)
        nc.sync.dma_start(dr["y"][s, mi], outT[:, mi, :])


# ---------------------------------------------------------------- execution
_CACHE = {}


def _get_nc():
    if "nc" not in _CACHE:
        _CACHE["nc"] = build_nc()
    return _CACHE["nc"]


def make_in_maps(x, consts):
    in_maps = []
    for c in range(N_CORES):
        xs = x[SPB * c:SPB * (c + 1)]  # (4, 1025, 256)
        xT = np.ascontiguousarray(xs.transpose(0, 2, 1)).reshape(SPB, 2, 128, NT + 1)
        in_maps.append({
            "xT": xT.astype(np.float32),
            "wqk": consts["wqk"], "wv": consts["wv"], "wp": consts["wp"],
            "expB1": np.ascontiguousarray(consts["expB1"]),
            "expB2": np.ascontiguousarray(consts["expB2"]),
            "sa": consts["sa"], "w3d": consts["w3d"],
            "zz": consts["zz"],
            "onesBD": np.ascontiguousarray(consts["onesBD"]),
            "zzb": np.ascontiguousarray(consts["zzb"]),
        })
    return in_maps


def assemble(results, consts):
    out = np.empty((B, NT + 1, C), np.float32)
    for c in range(N_CORES):
        y = results[c]["y"].reshape(SPB, 2, 128, NT + 1)  # (s, ci, p, t)
        yT = y.transpose(0, 3, 1, 2).reshape(SPB, NT + 1, C)  # (s, t, c)
        out[SPB * c:SPB * (c + 1), 0] = yT[:, NT] + consts["bias_cls"]
        out[SPB * c:SPB * (c + 1), 1:] = yT[:, :NT] + consts["bias_sp"]
    return out


def kernel(x, qkv_w, proj_w, proj_b, dwc_w, dwc_b,
           an_bias, ah_bias, aw_bias, na_bias, ha_bias, wa_bias):
    x = np.asarray(x, np.float32)
    consts = _host_consts(np.asarray(qkv_w, np.float32), np.asarray(proj_w, np.float32),
                          np.asarray(proj_b, np.float32), np.asarray(dwc_w, np.float32),
                          np.asarray(dwc_b, np.float32), np.asarray(an_bias, np.float32),
                          np.asarray(ah_bias, np.float32), np.asarray(aw_bias, np.float32),
                          np.asarray(na_bias, np.float32), np.asarray(ha_bias, np.float32),
                          np.asarray(wa_bias, np.float32))
    nc = _get_nc()
    in_maps = make_in_maps(x, consts)
    res = bass_utils.run_bass_kernel_spmd(nc, in_maps,
                                          core_ids=list(range(N_CORES)))
    return assemble(res.results, consts)

